# revision 12
# baseline (speedup 1.0000x reference)
"""Trainium2 Bass kernel for GQA MultiHeadAttention with ALiBi (B=2, S=2048,
D=1024, 16 Q heads / 4 KV heads, combined QKV projection, output projection).

Sharding (8 cores): core c -> (batch b = c//4, kv-group g = c%4) owning the 4
query heads 4g..4g+3 that share KV head g.  Wc is column-split, Wo row-split;
each core emits a partial [S, D] output and the host sums the 4 partials per
batch (+ bo).  No cross-core collectives.

Device-side layout is fully "transposed" so no on-device transposes of big
tensors are needed:
  qkvT[col, t] = Wc_sliceT @ xT           (host pre-transposes x)
  scoresT[k, q] = KT_kb^T @ QT_h          (per 128-row k-block, PSUM fp32)
  PT = exp(scoresT + pbias[k%128])        (ACT, per-partition fp32 bias)
  out'T[65, q] += (V|1)_kb^T * mscale_kb @ PT_kb   (unnormalized + denom row)
  outT = out'T[:64] * bcast(1/denom)
  y_partial[t, e] = outT^T @ Wo_slice     (per 128-row t-block)

The ALiBi bias slope*(i-j): the +slope*i row term cancels in softmax; -slope*j
is split into a per-partition part -slope*(j%128) (exact fp32 ACT bias) and a
per-k-block constant -slope*128*kb - BOUND folded multiplicatively into the
stationary V|ones tile scales (mscale, host-computed fp32, also carrying the
attention mask as 0/1).  BOUND=16 keeps exp() in range (true score max ~7.5).
"""

import os
from contextlib import ExitStack

import numpy as np
import ml_dtypes

import concourse.bass as bass
import concourse.tile as tile
import concourse.mybir as mybir
import concourse.bass_utils as bass_utils
from concourse import bacc

BF16 = ml_dtypes.bfloat16

D_MODEL, H, HKV, DK = 1024, 16, 4, 64
B, S = 2, 2048
QH_PER_CORE = 4          # query heads per core
FQ = QH_PER_CORE * DK    # 256 rows of outT / Wo slice per core
BOUND = 16.0
N_CORES = 8
KB = S // 128            # 16 k-blocks
TB = S // 128            # 16 t-blocks
QC = 4                   # q chunks of 512
QCW = 512

_nc_cache = {}


def _emit_kernel(nc, tensors):
    xt = tensors["xt"].ap()          # [8, 128, 2048] bf16  (c-chunk major xT)
    wc = tensors["wc"].ap()          # [128, 8*384] bf16
    bcb = tensors["bcb"].ap()        # [128, 3] f32
    wo = tensors["wo"].ap()          # [128, 2048] bf16 (f-chunk major Wo slice)
    ebias = tensors["ebias"].ap()    # [128, 64] f32 (full exp bias per hl,kb)
    ident = tensors["ident"].ap()    # [64, 64] bf16
    y = tensors["y"].ap()            # [16, 128, 1024] f32 out

    f32 = mybir.dt.float32
    bf16 = mybir.dt.bfloat16

    with tile.TileContext(nc) as tc, ExitStack() as big:
        sb = big.enter_context(tc.tile_pool(name="sb", bufs=1))

        # ---- persistent SBUF tensors ----
        xt_sb = sb.tile([128, 8, 2048], bf16, name="xt_sb")
        wc_sb = sb.tile([128, 8 * 384], bf16, name="wc_sb")
        bcb_sb = sb.tile([128, 3], f32, name="bcb_sb")
        wo_sb = sb.tile([128, 2048], bf16, name="wo_sb")
        ebias_sb = sb.tile([128, 64], f32, name="ebias_sb")
        ident_sb = sb.tile([64, 64], bf16, name="ident_sb")
        # qkvT: 3 col-blocks of 128: [Q heads 0-1 | Q heads 2-3 | K(0:64)+V(64:128)]
        qkvT = [sb.tile([128, 2048], bf16, name=f"qkvT{i}") for i in range(3)]
        # duplicated operands for 2x row-tiled scores matmuls: each head's QT
        # and the shared KT live in BOTH partition halves (T0: 0-63, T8:
        # 64-127), letting two 64-contraction matmuls run concurrently.
        kdup = sb.tile([128, 2048], bf16, name="kdup")
        qdup = [sb.tile([128, 2048], bf16, name=f"qdup{h}") for h in range(4)]
        vt = sb.tile([64, 2048], bf16, name="vt")
        oT1 = sb.tile([64, 2048], bf16, name="oT1")
        oT3 = sb.tile([64, 2048], bf16, name="oT3")
        vbase = sb.tile([128, KB * 65], bf16, name="vbase")       # V|ones per kb
        outT = [sb.tile([128, 2048], bf16, name=f"outT{i}") for i in range(2)]

        nc.scalar.dma_start(out=wc_sb, in_=wc)
        nc.gpsimd.dma_start(out=bcb_sb, in_=bcb)
        nc.gpsimd.dma_start(out=ebias_sb, in_=ebias)
        nc.gpsimd.dma_start(out=ident_sb, in_=ident)
        # x arrives t-chunk-major so the first QKV accumulation group (which
        # needs ALL c-chunks of one t-slice) can start after ~1MB; spread
        # across two DMA queues
        for t4 in range(4):
            for cc in range(8):
                eng = nc.sync if cc % 2 == 0 else nc.scalar
                eng.dma_start(
                    out=xt_sb[:, cc, t4 * 512:(t4 + 1) * 512],
                    in_=xt[cc][:, t4 * 512:(t4 + 1) * 512])
        nc.gpsimd.dma_start(out=wo_sb, in_=wo)

        nc.vector.memset(vbase, 1.0)

        # ---- stage A: qkvT = wc^T @ xT (+bc) ----
        with ExitStack() as st_a:
            psA = st_a.enter_context(
                tc.tile_pool(name="psA", bufs=2, space="PSUM"))
            psV = st_a.enter_context(
                tc.tile_pool(name="psV", bufs=2, space="PSUM"))
            # K/V first (stage B needs the full KT and V before any work on
            # head 0), then Q blocks; dup-DMAs issued per slice on the idle
            # Scalar engine's queue.
            for colb in (2, 0, 1):
                for t4 in range(4):
                    tsl = slice(t4 * 512, (t4 + 1) * 512)
                    ps = psA.tile([128, 512], f32, tag="psa")
                    for cc in range(8):
                        nc.tensor.matmul(
                            ps,
                            wc_sb[:, cc * 384 + colb * 128:
                                  cc * 384 + (colb + 1) * 128],
                            xt_sb[:, cc, tsl],
                            start=(cc == 0), stop=(cc == 7),
                        )
                    nc.vector.tensor_scalar_add(
                        qkvT[colb][:, tsl], ps, bcb_sb[:, colb:colb + 1])
                # consolidated dup DMAs: sync queue drains its xt share by
                # ~13us and is free; late heads (2,3) go on gpsimd
                if colb == 2:
                    nc.sync.dma_start(out=kdup[0:64, :], in_=qkvT[2][0:64, :])
                    nc.sync.dma_start(out=kdup[64:128, :],
                                      in_=qkvT[2][0:64, :])
                    nc.sync.dma_start(out=vt, in_=qkvT[2][64:128, :])
                    # transpose V into vbase cols per kb (PE stays busy)
                    for kb in range(KB):
                        pv = psV.tile([128, 64], bf16, tag="psv")
                        nc.tensor.transpose(
                            pv, vt[:, kb * 128:(kb + 1) * 128], ident_sb)
                        nc.vector.tensor_copy(
                            vbase[:, kb * 65:kb * 65 + 64], pv)
                else:
                    for hh in ((0, 1) if colb == 0 else (2, 3)):
                        src = qkvT[colb][(hh % 2) * 64:(hh % 2) * 64 + 64, :]
                        eng = nc.sync if colb == 0 else nc.gpsimd
                        eng.dma_start(out=qdup[hh][0:64, :], in_=src)
                        eng.dma_start(out=qdup[hh][64:128, :], in_=src)

        # ---- stage B: attention per (head-pair, q-chunk) ----
        with ExitStack() as st_b:
            psS = st_b.enter_context(
                tc.tile_pool(name="psS", bufs=3, space="PSUM"))
            psPV = st_b.enter_context(
                tc.tile_pool(name="psPV", bufs=2, space="PSUM"))
            ptp = st_b.enter_context(tc.tile_pool(name="ptp", bufs=6))
            dnp = st_b.enter_context(tc.tile_pool(name="dnp", bufs=4))
            dnd = st_b.enter_context(
                tc.tile_pool(name="dnd", bufs=4, space="DRAM"))

            # normalized outT destination: even heads write packed tiles
            # directly; odd heads go to aligned temps, DMA-shifted after.
            odst = {0: outT[0][0:64, :], 1: oT1[:, :],
                    2: outT[1][0:64, :], 3: oT3[:, :]}

            def normalize(hl, qc, opv):
                # Evacuate PV psum immediately (frees the bank / keeps PE
                # fed), then normalize from the SBUF copy: denom row
                # (partition 64) goes through a DRAM bounce (DMA crosses
                # partitions; DVE is lane-locked).
                pvs = dnp.tile([65, QCW], f32, tag="pvs", name="pvs")
                nc.vector.tensor_copy(pvs, opv)
                rcd = dnd.tile([1, QCW], f32, tag="rcd", name="rcd")
                nc.gpsimd.dma_start(out=rcd, in_=pvs[64:65, :])
                rbden = dnp.tile([64, QCW], f32, tag="rbden", name="rbden")
                nc.gpsimd.dma_start(out=rbden,
                                    in_=rcd.to_broadcast([64, QCW]))
                rb = dnp.tile([64, QCW], f32, tag="rb", name="rb")
                nc.vector.reciprocal_approx_fast(rb, rbden)
                nc.vector.tensor_mul(
                    odst[hl][:, qc * QCW:(qc + 1) * QCW], pvs[0:64, :], rb)

            for qcg in range(QC // 2):  # q-chunk pairs
                qc0, qc1 = 2 * qcg, 2 * qcg + 1
                for hl in range(4):
                    opv0 = psPV.tile([65, QCW], f32, tag="opv", name="opv0")
                    opv1 = psPV.tile([65, QCW], f32, tag="opv", name="opv1")
                    for kb in range(KB):
                        # scores: same kb on both row tiles — T0 computes
                        # qc0 (partitions 0-63), T8 computes qc1 (64-127)
                        # concurrently; the [128,1024] psum tile then has a
                        # kb-uniform bias so ONE FD=1024 exp covers it.
                        scs = psS.tile([128, 2 * QCW], f32, tag="scs",
                                       name="scs")
                        pt = ptp.tile([128, 2 * QCW], bf16, tag="pt",
                                      name="pt")
                        nc.tensor.matmul(
                            scs[:, 0:QCW],
                            kdup[0:64, kb * 128:(kb + 1) * 128],
                            qdup[hl][0:64, qc0 * QCW:(qc0 + 1) * QCW],
                            start=True, stop=True)
                        nc.tensor.matmul(
                            scs[:, QCW:2 * QCW],
                            kdup[64:128, kb * 128:(kb + 1) * 128],
                            qdup[hl][64:128, qc1 * QCW:(qc1 + 1) * QCW],
                            start=True, stop=True)
                        nc.scalar.activation(
                            pt, scs, mybir.ActivationFunctionType.Exp,
                            bias=ebias_sb[:, hl * 16 + kb:hl * 16 + kb + 1],
                            scale=1.0)
                        vb = vbase[:, kb * 65:(kb + 1) * 65]
                        nc.tensor.matmul(
                            opv0, vb, pt[:, 0:QCW],
                            start=(kb == 0), stop=(kb == KB - 1))
                        nc.tensor.matmul(
                            opv1, vb, pt[:, QCW:2 * QCW],
                            start=(kb == 0), stop=(kb == KB - 1))
                    normalize(hl, qc0, opv0)
                    normalize(hl, qc1, opv1)
                # this q-chunk-group's outT columns are final: shift the
                # odd heads into the packed tiles (overlaps with compute)
                sl = slice(qcg * 1024, (qcg + 1) * 1024)
                nc.scalar.dma_start(out=outT[0][64:128, sl], in_=oT1[:, sl])
                nc.scalar.dma_start(out=outT[1][64:128, sl], in_=oT3[:, sl])

        # ---- stage C: y_partial = outT^T @ wo ----
        with ExitStack() as st_c:
            psY = st_c.enter_context(
                tc.tile_pool(name="psY", bufs=6, space="PSUM"))
            ysb = st_c.enter_context(tc.tile_pool(name="ysb", bufs=6))
            dma_engs = [nc.sync, nc.scalar, nc.gpsimd]
            for tb in range(TB):
                yt = ysb.tile([128, 1024], bf16, tag="yt", name="yt")
                for eb in range(2):
                    py = psY.tile([128, 512], f32, tag="py", name="py")
                    for fc in range(2):
                        nc.tensor.matmul(
                            py,
                            outT[fc][:, tb * 128:(tb + 1) * 128],
                            wo_sb[:, fc * 1024 + eb * 512:
                                  fc * 1024 + (eb + 1) * 512],
                            start=(fc == 0), stop=(fc == 1),
                        )
                    if eb == 0:
                        nc.vector.tensor_copy(
                            yt[:, eb * 512:(eb + 1) * 512], py)
                    else:
                        nc.scalar.copy(yt[:, eb * 512:(eb + 1) * 512], py)
                dma_engs[tb % 3].dma_start(out=y[tb], in_=yt)


def _build():
    if "nc" in _nc_cache:
        return _nc_cache["nc"], _nc_cache["tensors"]
    nc = bacc.Bacc("TRN2", target_bir_lowering=False, debug=False,
                   enable_asserts=False, num_devices=N_CORES)
    bf16 = mybir.dt.bfloat16
    f32 = mybir.dt.float32
    tensors = {
        "xt": nc.dram_tensor("xt", [8, 128, 2048], bf16, kind="ExternalInput"),
        "wc": nc.dram_tensor("wc", [128, 8 * 384], bf16, kind="ExternalInput"),
        "bcb": nc.dram_tensor("bcb", [128, 3], f32, kind="ExternalInput"),
        "wo": nc.dram_tensor("wo", [128, 2048], bf16, kind="ExternalInput"),
        "ebias": nc.dram_tensor("ebias", [128, 64], f32,
                                kind="ExternalInput"),
        "ident": nc.dram_tensor("ident", [64, 64], bf16, kind="ExternalInput"),
        "y": nc.dram_tensor("y", [16, 128, 1024], bf16,
                            kind="ExternalOutput"),
    }
    _emit_kernel(nc, tensors)
    nc.compile()
    _nc_cache["nc"] = nc
    _nc_cache["tensors"] = tensors
    return nc, tensors


def _core_inputs(x, mask, Wc, bc, Wo, core):
    b, g = core // 4, core % 4
    heads = [QH_PER_CORE * g + i for i in range(QH_PER_CORE)]

    xT = np.ascontiguousarray(x[b].T)                      # [1024, 2048]
    xt = xT.reshape(8, 128, S).astype(BF16)

    q_cols = np.concatenate(
        [np.arange(h * DK, (h + 1) * DK) for h in heads])
    k_cols = np.arange(D_MODEL + g * DK, D_MODEL + (g + 1) * DK)
    v_cols = np.arange(D_MODEL + HKV * DK + g * DK,
                       D_MODEL + HKV * DK + (g + 1) * DK)
    cols = np.concatenate([q_cols, k_cols, v_cols])        # 384

    wcs = Wc[:, cols].astype(np.float32).copy()
    wcs[:, :FQ] /= np.sqrt(DK)
    # [1024, 384] -> [128, 8*384] (c-chunk major in free dim)
    wc_h = np.ascontiguousarray(
        wcs.reshape(8, 128, 384).transpose(1, 0, 2).reshape(128, 8 * 384)
    ).astype(BF16)

    bcs = bc[cols].astype(np.float32).copy()
    bcs[:FQ] /= np.sqrt(DK)
    bcb = np.ascontiguousarray(bcs.reshape(3, 128).T).astype(np.float32)

    wos = Wo[g * FQ:(g + 1) * FQ, :].astype(np.float32)    # [256, 1024]
    wo_h = np.ascontiguousarray(
        wos.reshape(2, 128, 1024).transpose(1, 0, 2).reshape(128, 2048)
    ).astype(BF16)

    # ebias[p, hl*16+kb] = -slope_hl*(kb*128+p) - BOUND (+ -1e9 where masked)
    slopes = np.array([2.0 ** (-(h + 1)) for h in heads], np.float64)
    p = np.arange(128, dtype=np.float64)
    kbv = np.arange(KB, dtype=np.float64)
    eb = (-slopes[None, :, None] * (kbv[None, None, :] * 128.0 + p[:, None, None])
          - BOUND)  # [128, 4, 16]
    mbad = (mask[b].reshape(KB, 128).T == 0)  # [128, 16]
    eb = eb + np.where(mbad, -1e9, 0.0)[:, None, :]
    ebias = np.ascontiguousarray(eb.reshape(128, 64)).astype(np.float32)

    ident = np.eye(64, dtype=np.float32).astype(BF16)

    return {
        "xt": xt, "wc": wc_h, "bcb": bcb, "wo": wo_h,
        "ebias": ebias, "ident": ident,
    }


def kernel(x, mask, Wc, bc, Wo, bo):
    x = np.asarray(x, np.float32)
    mask = np.asarray(mask)
    Wc = np.asarray(Wc, np.float32)
    bc = np.asarray(bc, np.float32)
    Wo = np.asarray(Wo, np.float32)
    bo = np.asarray(bo, np.float32)

    nc, tensors = _build()
    in_maps = [_core_inputs(x, mask, Wc, bc, Wo, c) for c in range(N_CORES)]
    trace = bool(int(os.environ.get("KERNEL_TRACE", "0")))
    try:
        res = bass_utils.run_bass_kernel_spmd(
            nc, in_maps, core_ids=list(range(N_CORES)), trace=trace)
    except ModuleNotFoundError:
        # profiling hook unavailable in this environment; run without trace
        res = bass_utils.run_bass_kernel_spmd(
            nc, in_maps, core_ids=list(range(N_CORES)), trace=False)
    _nc_cache["last_results"] = res

    y = np.zeros((B, S, D_MODEL), np.float32)
    for c in range(N_CORES):
        part = res.results[c]["y"].reshape(S, D_MODEL).astype(np.float32)
        y[c // 4] += part
    y += bo[None, None, :]
    return y
